# revision 1
# baseline (speedup 1.0000x reference)
"""Sparse (chunked-causal | bidirectional-block) GQA attention on 8 trn2 cores.

Full inputs in, full output out. Sharding: core j handles batch b = j // 4 and
kv-heads {2*(j%4), 2*(j%4)+1} (= query heads 4*(j%4) .. 4*(j%4)+3).

Per-core bass kernel (fp16 on-chip, fp32 PSUM/normalize):
  - q/k/v cast fp32->fp16 by whole-tensor contiguous SWDGE DMAs into DRAM
    staging (keeps Q7 descriptor generation cheap); K^T/Q^T via HWDGE xbar
    dma_start_transpose from staging; V (+ ones column) loaded strided from
    staging by HWDGE.
  - S^T[kv, q] per 128-kv-tile via PE matmul (lhsT = K^T tile, rhs = Q^T
    cols), packed into <=2-bank PSUM rounds (bufs=2 -> QK of round r+1
    overlaps exp of round r).
  - One ACT exp (scale=1/sqrt(D)) per round -> E (fp16, SBUF).
  - Partial 128x128 blocks masked multiplicatively on DVE (host-computed
    exact fp16 0/1 blocks; adjacent blocks merged into one op).
  - PV: per q-subtile accumulate matmuls lhsT=E-slice, rhs=V_aug (V with an
    appended ones column -> softmax denominators for free), own PSUM bank.
  - Normalize per group of 4 subtiles: one DVE reciprocal [128,4] + one
    broadcast tensor_tensor multiply.

The schedule (which 128x128 blocks exist / are masked) is computed on the host
from the actual bidirectional_mask + chunk_size, as the union over both batch
elements (the program is SPMD across cores); mask data stays exact per core.
"""

import math

import numpy as np

import concourse.bass as bass
import concourse.mybir as mybir
import concourse.tile as tile
from concourse import bacc
from concourse.bass_utils import run_bass_kernel_spmd

B, S, HQ, HKV, D = 2, 2048, 16, 8, 128
TS = 128                  # block tile size (partitions)
NT = S // TS              # 16 q/kv tiles
GROUP_SUBTILES = 4        # q-subtiles per group (512 q rows)
N_GROUPS = NT // GROUP_SUBTILES
ROUND_COLS_CAP = 1024     # 2 fp32 PSUM banks per round
BANK_COLS = 512           # fp32 cols per PSUM bank
N_CORES = 8
PAIRS_PER_CORE = 2        # kv heads per core
HEADS_PER_CORE = 4        # query heads per core

F16 = mybir.dt.float16
F32 = mybir.dt.float32


# ---------------------------------------------------------------- host masks

def _segment_ids(m):
    """[B, S] 0/1 -> contiguous-run segment ids (0 = not in a run)."""
    mm = m.astype(np.int64)
    padded = np.pad(mm, ((0, 0), (1, 0)))
    boundary = padded[:, 1:] > padded[:, :-1]
    return mm * np.cumsum(boundary, axis=1)


def _allowed_T(bidirectional_mask, chunk):
    """Per-batch allowed mask, transposed: [B, S(kv), S(q)] bool."""
    seg = _segment_ids(np.asarray(bidirectional_mask))
    r = np.arange(S)
    chunk_ok = (r[:, None] // chunk == r[None, :] // chunk) & (r[:, None] >= r[None, :])
    out = np.zeros((B, S, S), dtype=bool)
    for b in range(B):
        bid = (seg[b][:, None] == seg[b][None, :]) & (seg[b][:, None] > 0)
        out[b] = (chunk_ok | bid).T
    return out


class Schedule:
    """Static (union-over-batch) block schedule, shared by all 8 cores."""

    def __init__(self, allowed_T):
        blocks = allowed_T.reshape(B, NT, TS, NT, TS)
        b_any = blocks.any(axis=(2, 4))   # [B, t, s]
        b_all = blocks.all(axis=(2, 4))
        self.u_any = b_any.any(axis=0)    # [t, s]
        self.u_all = b_all.all(axis=0)
        self.partial = self.u_any & ~self.u_all

        self.mask_blocks = []             # list of (t, s) in fixed order
        mask_idx = {}

        # groups[g] = list of rounds; round = dict with fields:
        #   cols: total packed columns
        #   qk: list of (t, coff, q0, n)            matmul pieces
        #   masks: list of (e_off, midx, nblk)      merged DVE mask multiplies
        #   pv: {s_local: [(t, e_off), ...]}        accumulation lists
        self.groups = []
        for g in range(N_GROUPS):
            s0, s1 = g * GROUP_SUBTILES, (g + 1) * GROUP_SUBTILES
            t_entries = []
            for t in range(NT):
                ss = [s for s in range(s0, s1) if self.u_any[t, s]]
                if not ss:
                    continue
                lo, hi = min(ss), max(ss) + 1
                t_entries.append((t, lo, hi))

            rounds = []
            cur = None
            for (t, lo, hi) in t_entries:
                ncols = (hi - lo) * TS
                if cur is None or cur["cols"] + ncols > ROUND_COLS_CAP:
                    cur = {"cols": 0, "qk": [], "raw_masks": [],
                           "pv": {sl: [] for sl in range(GROUP_SUBTILES)}}
                    rounds.append(cur)
                toff = cur["cols"]
                # split matmul pieces at PSUM bank boundaries
                q0 = lo * TS
                off = toff
                rem = ncols
                while rem > 0:
                    n = min(BANK_COLS - off % BANK_COLS, rem)
                    cur["qk"].append((t, off, q0, n))
                    off += n
                    q0 += n
                    rem -= n
                for s in range(lo, hi):
                    if not self.u_any[t, s]:
                        continue
                    e_off = toff + (s - lo) * TS
                    if self.partial[t, s]:
                        if (t, s) not in mask_idx:
                            mask_idx[(t, s)] = len(self.mask_blocks)
                            self.mask_blocks.append((t, s))
                        cur["raw_masks"].append((e_off, mask_idx[(t, s)]))
                    cur["pv"][s - s0].append((t, e_off))
                cur["cols"] += ncols

            # merge adjacent mask multiplies (contiguous e cols + mask idxs)
            for rnd in rounds:
                merged = []
                for (e_off, midx) in sorted(rnd.pop("raw_masks")):
                    if (merged and merged[-1][0] + merged[-1][2] * TS == e_off
                            and merged[-1][1] + merged[-1][2] == midx):
                        merged[-1][2] += 1
                    else:
                        merged.append([e_off, midx, 1])
                rnd["masks"] = [tuple(x) for x in merged]
            self.groups.append(rounds)

        self.n_masks = len(self.mask_blocks)

    def mask_data(self, allowed_T_b):
        """[TS, n_masks, TS] fp16 0/1 blocks (partition-major) for one batch."""
        out = np.zeros((TS, max(self.n_masks, 1), TS), dtype=np.float16)
        for i, (t, s) in enumerate(self.mask_blocks):
            out[:, i, :] = allowed_T_b[t * TS:(t + 1) * TS, s * TS:(s + 1) * TS]
        return out

    def key(self):
        return (self.u_any.tobytes(), self.u_all.tobytes())


# ------------------------------------------------------------- kernel build

def _broadcast_free(ap, n):
    """Append a 0-step free dim of size n to an AP (read-broadcast)."""
    return bass.AP(tensor=ap.tensor, offset=ap.offset, ap=[*ap.ap, [0, n]])


def _build_body(nc, tc, sched: Schedule, tensors, safe_pv=False):
    q_in, k_in, v_in, m_in, o_out = tensors
    scale = 1.0 / math.sqrt(D)
    ctxs = []
    pv_first_mms = []   # (first_inst_name, [other_inst_names]) per PSUM bank

    def pool(*a, **kw):
        p = tc.tile_pool(*a, **kw)
        ctxs.append(p)
        return p.__enter__()

    dstage = pool(name="dram_stage", bufs=1, space="DRAM")
    consts = pool(name="consts", bufs=1)
    ktp = pool(name="ktp", bufs=2)
    qtp = pool(name="qtp", bufs=4)
    vp = pool(name="vp", bufs=2)
    epool = pool(name="epool", bufs=5)
    outp = pool(name="outp", bufs=3)
    small = pool(name="small", bufs=4)
    stp = pool(name="st_psum", bufs=2, space="PSUM")
    pvp = pool(name="pv_psum", bufs=1 if safe_pv else 2, space="PSUM")

    # fp16 staging for the transposes (SWDGE casts: K whole, Q per head-pair
    # so the first transpose can start early). V is cast straight into SBUF.
    k16 = dstage.tile([S, PAIRS_PER_CORE, D], F16, tag="k16")
    q16a = dstage.tile([S, 2, D], F16, tag="q16a")
    q16b = dstage.tile([S, 2, D], F16, tag="q16b")
    nc.gpsimd.dma_start(out=k16, in_=k_in[:, :, :])
    nc.gpsimd.dma_start(out=q16a, in_=q_in[:, 0:2, :])
    q16s = [q16a, q16b]

    nmask = max(sched.n_masks, 1)
    mask_sb = consts.tile([TS, nmask, TS], F16)

    # stage everything up-front, ordered so the first head's K^T/Q^T are
    # ready as early as possible; the scheduler overlaps the rest
    kts, qts, vas = [], [], []

    kt0 = ktp.tile([TS, S], F16, tag="kt")
    nc.sync.dma_start_transpose(out=kt0, in_=k16[:, 0, :])
    kts.append(kt0)
    qt0 = qtp.tile([TS, S], F16, tag="qt")
    nc.sync.dma_start_transpose(out=qt0, in_=q16a[:, 0, :])
    qts.append(qt0)

    nc.gpsimd.dma_start(out=q16b, in_=q_in[:, 2:4, :])

    v_aug0 = vp.tile([TS, NT, D + 4], F16, tag="vaug")
    nc.gpsimd.dma_start(
        out=v_aug0[:, :, 0:D],
        in_=v_in[:, 0, :].rearrange("(t p) d -> p t d", p=TS),
    )
    nc.vector.memset(v_aug0[:, :, D:D + 1], 1.0)
    vas.append(v_aug0)

    nc.scalar.dma_start(out=mask_sb, in_=m_in[:, :, :])

    qt1 = qtp.tile([TS, S], F16, tag="qt")
    nc.sync.dma_start_transpose(out=qt1, in_=q16a[:, 1, :])
    qts.append(qt1)

    kt1 = ktp.tile([TS, S], F16, tag="kt")
    nc.sync.dma_start_transpose(out=kt1, in_=k16[:, 1, :])
    kts.append(kt1)

    v_aug1 = vp.tile([TS, NT, D + 4], F16, tag="vaug")
    nc.gpsimd.dma_start(
        out=v_aug1[:, :, 0:D],
        in_=v_in[:, 1, :].rearrange("(t p) d -> p t d", p=TS),
    )
    nc.vector.memset(v_aug1[:, :, D:D + 1], 1.0)
    vas.append(v_aug1)

    for h in (2, 3):
        qt = qtp.tile([TS, S], F16, tag="qt")
        nc.sync.dma_start_transpose(out=qt, in_=q16b[:, h - 2, :])
        qts.append(qt)

    # flatten all (head, group, round) work items for software-pipelined
    # emission: PV/normalize lag one round behind QK/exp/mask so the PE
    # stream never waits on exp of the round it just produced
    nbank = GROUP_SUBTILES if safe_pv else 2
    per = 1 if safe_pv else 2
    work = []
    for pair in range(PAIRS_PER_CORE):
        for g_head in range(2):
            head = 2 * pair + g_head
            for g in range(N_GROUPS):
                for ri, rnd in enumerate(sched.groups[g]):
                    work.append({
                        "head": head, "pair": pair, "g": g, "rnd": rnd,
                        "first": ri == 0,
                        "last": ri == len(sched.groups[g]) - 1,
                    })

    group_state = {}

    def emit_front(w):
        st = stp.tile([TS, ROUND_COLS_CAP], F32, tag="st")
        kt, qt = kts[w["pair"]], qts[w["head"]]
        for (t, coff, q0, n) in w["rnd"]["qk"]:
            nc.tensor.matmul(
                st[:, coff:coff + n],
                lhsT=kt[:, t * TS:(t + 1) * TS],
                rhs=qt[:, q0:q0 + n],
                start=True, stop=True,
            )
        e = epool.tile([TS, ROUND_COLS_CAP], F16, tag="e")
        nc.scalar.activation(
            e[:, 0:w["rnd"]["cols"]], st[:, 0:w["rnd"]["cols"]],
            mybir.ActivationFunctionType.Exp, scale=scale,
        )
        for (e_off, midx, nblk) in w["rnd"]["masks"]:
            width = nblk * TS
            nc.vector.tensor_mul(
                e[:, e_off:e_off + width],
                e[:, e_off:e_off + width],
                mask_sb[:, midx:midx + nblk, :],
            )
        w["e"] = e

    def emit_back(w):
        g, head = w["g"], w["head"]
        if w["first"]:
            gs = {
                "pv": pvp.tile([TS, nbank, per, BANK_COLS // per], F32,
                               name=f"pv_{head}_{g}", tag="pv"),
                "bank_first": [None] * nbank,
                "bank_mms": [[] for _ in range(nbank)],
                "bank_total": [0] * nbank,
                "bank_done": [0] * nbank,
            }
            for r in sched.groups[g]:
                for sl in range(GROUP_SUBTILES):
                    gs["bank_total"][sl // per] += len(r["pv"][sl])
            group_state[(head, g)] = gs
        gs = group_state[(head, g)]
        pv, e, v_aug = gs["pv"], w["e"], vas[w["pair"]]
        for sl in range(GROUP_SUBTILES):
            bk, sub = divmod(sl, per)
            for (t, e_off) in w["rnd"]["pv"][sl]:
                first = gs["bank_first"][bk] is None
                gs["bank_done"][bk] += 1
                mm = nc.tensor.matmul(
                    pv[:, bk, sub, 0:D + 1],
                    lhsT=e[:, e_off:e_off + TS],
                    rhs=v_aug[:, t, 0:D + 1],
                    start=first,
                    stop=gs["bank_done"][bk] == gs["bank_total"][bk],
                )
                if first:
                    gs["bank_first"][bk] = mm.ins.name
                else:
                    gs["bank_mms"][bk].append(mm.ins.name)
        if not w["last"]:
            return
        pv_first_mms.extend(
            (f, o) for f, o in zip(gs["bank_first"], gs["bank_mms"])
            if f is not None)
        recip = small.tile([TS, nbank, per], F32, tag="recip")
        nc.vector.reciprocal(recip, pv[:, :, :, D])
        out_sb = outp.tile([TS, nbank, per, D], F16, tag="outsb")
        nc.vector.tensor_mul(out_sb, pv[:, :, :, 0:D],
                             _broadcast_free(recip, D))
        rows = GROUP_SUBTILES * TS
        nc.sync.dma_start(
            out=o_out[g * rows:(g + 1) * rows, head, :]
                .rearrange("(t p) d -> p t d", p=TS),
            in_=out_sb,
        )

    LAG = min(3, max(1, len(work) - 1))
    for i, w in enumerate(work):
        emit_front(w)
        if i >= LAG:
            emit_back(work[i - LAG])
    for w in work[len(work) - LAG:]:
        emit_back(w)

    for p in reversed(ctxs):
        p.__exit__(None, None, None)
    return pv_first_mms


def _verify_pv_order(nc, pv_first_mms):
    """Each PSUM bank's start=True matmul must precede its other matmuls in
    the final (scheduled) program order."""
    pos = {}
    i = 0
    for bb in nc.m.functions[0].blocks:
        for ins in bb.instructions:
            pos[ins.name] = i
            i += 1
    for first, others in pv_first_mms:
        p0 = pos.get(first)
        if p0 is None:
            return False
        for o in others:
            po = pos.get(o)
            if po is None or po < p0:
                return False
    return True


def _build_kernel(sched: Schedule, repeat: int = 1, safe_pv: bool = False):
    nc = bacc.Bacc("TRN2", target_bir_lowering=False, debug=False,
                   num_devices=N_CORES, name="sparse_attn")

    q_in = nc.dram_tensor("q_sh", [S, HEADS_PER_CORE, D], F32, kind="ExternalInput")
    k_in = nc.dram_tensor("k_sh", [S, PAIRS_PER_CORE, D], F32, kind="ExternalInput")
    v_in = nc.dram_tensor("v_sh", [S, PAIRS_PER_CORE, D], F32, kind="ExternalInput")
    m_in = nc.dram_tensor("masks", [TS, max(sched.n_masks, 1), TS], F16,
                          kind="ExternalInput")
    o_out = nc.dram_tensor("o_sh", [S, HEADS_PER_CORE, D], F16,
                           kind="ExternalOutput")
    tensors = (q_in, k_in, v_in, m_in, o_out)

    with tile.TileContext(nc) as tc:
        if repeat == 1:
            pv_first_mms = _build_body(nc, tc, sched, tensors, safe_pv=safe_pv)
        else:
            acc = []

            def body(iv, unroll=1):
                acc.extend(_build_body(nc, tc, sched, tensors, safe_pv=safe_pv))
            with tc.For_i(0, repeat, 1) as _i:
                body(_i)
            pv_first_mms = acc

    nc.compile()
    if not safe_pv and not _verify_pv_order(nc, pv_first_mms):
        return _build_kernel(sched, repeat=repeat, safe_pv=True)
    return nc


# --------------------------------------------------------------- entry point

_CACHE = {}


def _get_kernel(sched: Schedule, repeat: int = 1):
    key = (sched.key(), repeat)
    if key not in _CACHE:
        _CACHE[key] = _build_kernel(sched, repeat)
    return _CACHE[key]


def _shard_inputs(q, k, v, masks_f16):
    in_maps = []
    for core in range(N_CORES):
        b = core // 4
        m = core % 4
        in_maps.append({
            "q_sh": np.ascontiguousarray(q[b, :, 4 * m:4 * m + 4, :]),
            "k_sh": np.ascontiguousarray(k[b, :, 2 * m:2 * m + 2, :]),
            "v_sh": np.ascontiguousarray(v[b, :, 2 * m:2 * m + 2, :]),
            "masks": masks_f16[b],
        })
    return in_maps


def kernel(q, k, v, bidirectional_mask, chunk_size):
    q = np.asarray(q, dtype=np.float32)
    k = np.asarray(k, dtype=np.float32)
    v = np.asarray(v, dtype=np.float32)
    chunk = int(np.asarray(chunk_size))

    allowed_T = _allowed_T(bidirectional_mask, chunk)
    sched = Schedule(allowed_T)
    nc = _get_kernel(sched)

    masks_f16 = [sched.mask_data(allowed_T[b]) for b in range(B)]
    in_maps = _shard_inputs(q, k, v, masks_f16)

    res = run_bass_kernel_spmd(nc, in_maps, list(range(N_CORES)))

    out = np.empty((B, S, HQ, D), dtype=np.float32)
    for core in range(N_CORES):
        b = core // 4
        m = core % 4
        out[b, :, 4 * m:4 * m + 4, :] = res.results[core]["o_sh"].astype(np.float32)
    return out



# revision 4
# speedup vs baseline: 1.4745x; 1.4745x over previous
"""Sparse (chunked-causal | bidirectional-block) GQA attention on 8 trn2 cores.

Full inputs in, full output out. Sharding: core j handles batch b = j // 4 and
kv-heads {2*(j%4), 2*(j%4)+1} (= query heads 4*(j%4) .. 4*(j%4)+3).

The host does all layout work so the device kernel is pure attention math on
DMA-friendly layouts:
  - q/k cast to fp16 (q pre-scaled by 1/sqrt(D)) and pre-transposed to
    [d, s] so QK^T needs no on-device transposes; v cast to fp16 with a ones
    column appended (softmax denominators fall out of the PV matmul).
  - all device inputs/outputs are laid out so every DMA descriptor is >=4KB
    contiguous per partition.
  - the block schedule (which 128x128 blocks exist, trimmed to their true
    column extent) is computed from the actual mask as the union over both
    batch elements (SPMD: one program for all 8 cores); mask data stays
    exact per core/batch.

Per-core bass kernel, per (head, group-of-512-q):
  - S^T[kv, q] via PE matmuls (lhsT = K^T slice, rhs = Q^T cols) into a
    3-bank PSUM tile (pieces split at bank boundaries), fp16 in / fp32 out.
  - one ACT exp per group -> E (fp16, SBUF).
  - partial blocks are packed at the tail of the group's columns, so ONE
    DVE multiply with the (host-packed, batch-exact) 0/1 mask handles them.
  - PV: per block, accumulate matmul lhsT=E-slice, rhs=V_aug tile into a
    2-bank PSUM group tile; ones column gives denominators.
  - normalize: one DVE reciprocal + one broadcast multiply into a
    4-head-interleaved out tile; one output DMA per group of 512 q rows.
"""

import math

import numpy as np

import concourse.bass as bass
import concourse.mybir as mybir
import concourse.tile as tile
from concourse import bacc
from concourse.bass_utils import run_bass_kernel_spmd

B, S, HQ, HKV, D = 2, 2048, 16, 8, 128
TS = 128                  # block tile size (partitions)
NT = S // TS              # 16 q/kv tiles
GROUP_SUBTILES = 4        # q-subtiles per group (512 q rows)
N_GROUPS = NT // GROUP_SUBTILES
BANK_COLS = 512           # fp32 cols per PSUM bank
ST_COLS = 1536            # st tile cols (3 banks; one group fits in one round)
N_CORES = 8
PAIRS_PER_CORE = 2        # kv heads per core
HEADS_PER_CORE = 4        # query heads per core

F16 = mybir.dt.float16
F32 = mybir.dt.float32


# ---------------------------------------------------------------- host masks

def _segment_ids(m):
    """[B, S] 0/1 -> contiguous-run segment ids (0 = not in a run)."""
    mm = m.astype(np.int64)
    padded = np.pad(mm, ((0, 0), (1, 0)))
    boundary = padded[:, 1:] > padded[:, :-1]
    return mm * np.cumsum(boundary, axis=1)


def _allowed_T(bidirectional_mask, chunk):
    """Per-batch allowed mask, transposed: [B, S(kv), S(q)] bool."""
    seg = _segment_ids(np.asarray(bidirectional_mask))
    r = np.arange(S)
    chunk_ok = (r[:, None] // chunk == r[None, :] // chunk) & (r[:, None] >= r[None, :])
    out = np.zeros((B, S, S), dtype=bool)
    for b in range(B):
        bid = (seg[b][:, None] == seg[b][None, :]) & (seg[b][:, None] > 0)
        out[b] = (chunk_ok | bid).T
    return out


class Schedule:
    """Static (union-over-batch) trimmed block schedule, shared by all cores.

    groups[g] = work dict with fields:
      cols: total packed e-columns for the group
      qk:   [(t, e_off, q_abs, n)]        matmul pieces (bank-split)
      mask: (e_lo, mbuf_off, w) or None   single DVE mask multiply
      pv:   {s_local: [(t, e_off, w, p_lo, full)]}  accumulation lists
    """

    def __init__(self, allowed_T):
        blocks = allowed_T.reshape(B, NT, TS, NT, TS)
        b_any = blocks.any(axis=(2, 4))   # [B, t, s]
        b_all = blocks.all(axis=(2, 4))
        self.u_any = b_any.any(axis=0)    # [t, s]
        self.u_all = b_all.all(axis=0)
        self.partial = self.u_any & ~self.u_all
        colmask = blocks.any(axis=(0, 2))  # [t, s, q_in_tile]
        qlo = np.zeros((NT, NT), np.int64)
        qhi = np.zeros((NT, NT), np.int64)
        for t in range(NT):
            for s in range(NT):
                if not self.u_any[t, s]:
                    continue
                c = colmask[t, s]
                lo = int(np.argmax(c))
                hi = TS - int(np.argmax(c[::-1]))
                # snap to a PE-tile-aligned window (out base partition of the
                # PV matmul must be 0/32/64/96 for <=32 rows, 0/64 for <=64)
                if (lo // 32) * 32 + 32 >= hi:
                    lo = (lo // 32) * 32
                    hi = lo + 32
                elif (lo // 64) * 64 + 64 >= hi:
                    lo = (lo // 64) * 64
                    hi = lo + 64
                else:
                    lo, hi = 0, TS
                qlo[t, s] = lo
                qhi[t, s] = hi
        self.qlo, self.qhi = qlo, qhi

        self.mask_slices = []   # ordered (t, abs_lo, abs_hi) -> host buffer cols
        mbuf_off = 0
        self.groups = []
        for g in range(N_GROUPS):
            s0 = g * GROUP_SUBTILES
            # per-t merged segments of contiguous same-partiality blocks
            entries = []
            for t in range(NT):
                blks = [(s, qlo[t, s], qhi[t, s])
                        for s in range(s0, s0 + GROUP_SUBTILES) if self.u_any[t, s]]
                if not blks:
                    continue
                segs = []  # [abs_lo, abs_hi, partial, [(s, abs_lo, abs_hi)]]
                for (s, lo_, hi_) in blks:
                    al, ah = s * TS + lo_, s * TS + hi_
                    p = bool(self.partial[t, s])
                    if segs and segs[-1][2] == p and segs[-1][1] == al:
                        segs[-1][1] = ah
                        segs[-1][3].append((s, al, ah))
                    else:
                        segs.append([al, ah, p, [(s, al, ah)]])
                entries.append((t, segs))

            work = {"cols": 0, "qk": [], "mask": None,
                    "pv": {sl: [] for sl in range(GROUP_SUBTILES)}}
            full_list = [(t, seg) for (t, segs) in entries for seg in segs if not seg[2]]
            part_list = [(t, seg) for (t, segs) in entries for seg in segs if seg[2]]
            off = 0
            for (t, (lo, hi, p, sblks)) in full_list + part_list:
                w = hi - lo
                o, q0, rem = off, lo, w
                while rem > 0:
                    n = min(BANK_COLS - o % BANK_COLS, rem)
                    work["qk"].append((t, o, q0, n))
                    o += n
                    q0 += n
                    rem -= n
                for (s, bl, bh) in sblks:
                    e_off = off + (bl - lo)
                    bw = bh - bl
                    work["pv"][s - s0].append(
                        (t, e_off, bw, bl - s * TS, bw == TS))
                if p:
                    for (s, bl, bh) in sblks:
                        self.mask_slices.append((t, bl, bh))
                off += w
            part_w = sum(seg[1] - seg[0] for (_, seg) in part_list)
            if part_w:
                work["mask"] = (off - part_w, mbuf_off, part_w)
                mbuf_off += part_w
            work["cols"] = off
            assert off <= ST_COLS, f"group {g} cols {off} > {ST_COLS}"
            # full-width blocks first within each subtile list (bank arming)
            for sl in range(GROUP_SUBTILES):
                work["pv"][sl].sort(key=lambda x: (0 if x[4] else 1,))
            self.groups.append(work)

        self.n_mask_cols = mbuf_off

    def mask_data(self, allowed_T_b):
        """[TS, n_mask_cols] fp16 0/1 packed mask buffer for one batch."""
        out = np.zeros((TS, max(self.n_mask_cols, 1)), dtype=np.float16)
        off = 0
        for (t, bl, bh) in self.mask_slices:
            w = bh - bl
            out[:, off:off + w] = allowed_T_b[t * TS:(t + 1) * TS, bl:bh]
            off += w
        return out

    def key(self):
        return (self.u_any.tobytes(), self.u_all.tobytes(),
                self.qlo.tobytes(), self.qhi.tobytes())


# ------------------------------------------------------------- kernel build

def _broadcast_free(ap, n):
    """Append a 0-step free dim of size n to an AP (read-broadcast)."""
    return bass.AP(tensor=ap.tensor, offset=ap.offset, ap=[*ap.ap, [0, n]])


def _split_dim(ap, n0, n1):
    """Split an AP's first free dim of size n0*n1 into (n0, n1)."""
    (pstep, pnum), (fstep, fnum), *rest = ap.ap
    assert fnum == n0 * n1
    return bass.AP(tensor=ap.tensor, offset=ap.offset,
                   ap=[[pstep, pnum], [fstep * n1, n0], [fstep, n1], *rest])


def _build_body(nc, tc, sched: Schedule, tensors, safe_pv=False):
    qT_in, kT_in, v_in, m_in, o_out = tensors
    ctxs = []
    pv_first_mms = []   # (first_inst_name, [other_inst_names]) per PSUM bank

    def pool(*a, **kw):
        p = tc.tile_pool(*a, **kw)
        ctxs.append(p)
        return p.__enter__()

    consts = pool(name="consts", bufs=1)
    ktp = pool(name="ktp", bufs=2)
    qtp = pool(name="qtp", bufs=4)
    vp = pool(name="vp", bufs=1)
    epool = pool(name="epool", bufs=4)
    outp = pool(name="outp", bufs=N_GROUPS)
    small = pool(name="small", bufs=4)
    stp = pool(name="st_psum", bufs=1 if safe_pv else 2, space="PSUM")
    pvp = pool(name="pv_psum", bufs=1, space="PSUM")

    nmask = max(sched.n_mask_cols, 1)
    mask_sb = consts.tile([TS, nmask], F16)

    # loads, ordered so head 0's operands and the mask/v tiles land first
    kts, qts = [], []
    kt0 = ktp.tile([TS, S], F16, tag="kt")
    nc.sync.dma_start(out=kt0, in_=kT_in[:, 0, :])
    kts.append(kt0)
    qt0 = qtp.tile([TS, S], F16, tag="qt")
    nc.sync.dma_start(out=qt0, in_=qT_in[:, 0, :])
    qts.append(qt0)
    nc.sync.dma_start(out=mask_sb, in_=m_in[:, :])
    v_sb = vp.tile([TS, NT, PAIRS_PER_CORE, D + 1], F16, tag="v")
    nc.sync.dma_start(out=v_sb, in_=v_in[:, :, :, :])
    qt1 = qtp.tile([TS, S], F16, tag="qt")
    nc.sync.dma_start(out=qt1, in_=qT_in[:, 1, :])
    qts.append(qt1)
    kt1 = ktp.tile([TS, S], F16, tag="kt")
    nc.sync.dma_start(out=kt1, in_=kT_in[:, 1, :])
    kts.append(kt1)
    for h in (2, 3):
        qt = qtp.tile([TS, S], F16, tag="qt")
        nc.sync.dma_start(out=qt, in_=qT_in[:, h, :])
        qts.append(qt)

    out_tiles = [outp.tile([TS, GROUP_SUBTILES, HEADS_PER_CORE, D], F16,
                           name=f"out_{g}", tag="out")
                 for g in range(N_GROUPS)]

    nbank = GROUP_SUBTILES if safe_pv else 2
    per = 1 if safe_pv else 2

    # work items: head-major, group-minor; PV/normalize lag behind QK/exp/mask
    work = []
    for pair in range(PAIRS_PER_CORE):
        for g_head in range(2):
            head = 2 * pair + g_head
            for g in range(N_GROUPS):
                work.append({"head": head, "pair": pair, "g": g,
                             "w": sched.groups[g]})

    def emit_front(w):
        gw = w["w"]
        st = stp.tile([TS, ST_COLS], F32, tag="st")
        kt, qt = kts[w["pair"]], qts[w["head"]]
        for (t, e_off, q0, n) in gw["qk"]:
            nc.tensor.matmul(
                st[:, e_off:e_off + n],
                lhsT=kt[:, t * TS:(t + 1) * TS],
                rhs=qt[:, q0:q0 + n],
                start=True, stop=True,
            )
        e = epool.tile([TS, ST_COLS], F16, tag="e")
        nc.scalar.activation(
            e[:, 0:gw["cols"]], st[:, 0:gw["cols"]],
            mybir.ActivationFunctionType.Exp,
        )
        if gw["mask"] is not None:
            (e_lo, moff, mw) = gw["mask"]
            nc.vector.tensor_mul(
                e[:, e_lo:e_lo + mw],
                e[:, e_lo:e_lo + mw],
                mask_sb[:, moff:moff + mw],
            )
        w["e"] = e

    def emit_back(w):
        gw, g, head, pair = w["w"], w["g"], w["head"], w["pair"]
        pv = pvp.tile([TS, nbank, per, BANK_COLS // per], F32,
                      name=f"pv_{head}_{g}", tag="pv")
        e = w["e"]
        bank_first = [None] * nbank
        bank_mms = [[] for _ in range(nbank)]
        bank_total = [0] * nbank
        bank_done = [0] * nbank
        for sl in range(GROUP_SUBTILES):
            bank_total[sl // per] += len(gw["pv"][sl])
        for sl in range(GROUP_SUBTILES):
            bk, sub = divmod(sl, per)
            for (t, e_off, bw, p_lo, full) in gw["pv"][sl]:
                first = bank_first[bk] is None
                assert not first or full, "bank must be armed by a full block"
                bank_done[bk] += 1
                mm = nc.tensor.matmul(
                    pv[p_lo:p_lo + bw, bk, sub, 0:D + 1],
                    lhsT=e[:, e_off:e_off + bw],
                    rhs=v_sb[:, t, pair, 0:D + 1],
                    start=first,
                    stop=bank_done[bk] == bank_total[bk],
                    tile_position=(0, p_lo),
                )
                if first:
                    bank_first[bk] = mm.ins.name
                else:
                    bank_mms[bk].append(mm.ins.name)
        pv_first_mms.extend(
            (f, o) for f, o in zip(bank_first, bank_mms) if f is not None)

        recip = small.tile([TS, nbank, per], F32, tag="recip")
        nc.vector.reciprocal(recip, pv[:, :, :, D])
        out_t = out_tiles[g]
        out_ap = _split_dim(out_t[:, :, head, :], nbank, per)
        nc.vector.tensor_mul(out_ap, pv[:, :, :, 0:D],
                             _broadcast_free(recip, D))
        if head == HEADS_PER_CORE - 1:
            rows = GROUP_SUBTILES * TS
            nc.sync.dma_start(out=o_out[:, g, :, :, :], in_=out_t)

    LAG = min(2, max(1, len(work) - 1))
    for i, w in enumerate(work):
        emit_front(w)
        if i >= LAG:
            emit_back(work[i - LAG])
    for w in work[len(work) - LAG:]:
        emit_back(w)

    for p in reversed(ctxs):
        p.__exit__(None, None, None)
    return pv_first_mms


def _verify_pv_order(nc, pv_first_mms):
    """Each PSUM bank's start=True matmul must precede its other matmuls in
    the final (scheduled) program order."""
    pos = {}
    i = 0
    for bb in nc.m.functions[0].blocks:
        for ins in bb.instructions:
            pos[ins.name] = i
            i += 1
    for first, others in pv_first_mms:
        p0 = pos.get(first)
        if p0 is None:
            return False
        for o in others:
            po = pos.get(o)
            if po is None or po < p0:
                return False
    return True


def _build_kernel(sched: Schedule, safe_pv: bool = False):
    nc = bacc.Bacc("TRN2", target_bir_lowering=False, debug=False,
                   num_devices=N_CORES, name="sparse_attn")

    qT_in = nc.dram_tensor("qT", [TS, HEADS_PER_CORE, S], F16, kind="ExternalInput")
    kT_in = nc.dram_tensor("kT", [TS, PAIRS_PER_CORE, S], F16, kind="ExternalInput")
    v_in = nc.dram_tensor("vaug", [TS, NT, PAIRS_PER_CORE, D + 1], F16,
                          kind="ExternalInput")
    m_in = nc.dram_tensor("maskb", [TS, max(sched.n_mask_cols, 1)], F16,
                          kind="ExternalInput")
    o_out = nc.dram_tensor("o", [TS, N_GROUPS, GROUP_SUBTILES, HEADS_PER_CORE, D],
                           F16, kind="ExternalOutput")
    tensors = (qT_in, kT_in, v_in, m_in, o_out)

    with tile.TileContext(nc) as tc:
        pv_first_mms = _build_body(nc, tc, sched, tensors, safe_pv=safe_pv)

    nc.compile()
    if not safe_pv and not _verify_pv_order(nc, pv_first_mms):
        return _build_kernel(sched, safe_pv=True)
    return nc


# --------------------------------------------------------------- entry point

_CACHE = {}


def _get_kernel(sched: Schedule):
    key = sched.key()
    if key not in _CACHE:
        _CACHE[key] = _build_kernel(sched)
    return _CACHE[key]


def _shard_inputs(q, k, v, masks_f16):
    scale = 1.0 / math.sqrt(D)
    in_maps = []
    for core in range(N_CORES):
        b = core // 4
        m = core % 4
        qT = np.ascontiguousarray(
            (q[b, :, 4 * m:4 * m + 4, :] * scale).astype(np.float16)
            .transpose(2, 1, 0))                       # [D, 4, S]
        kT = np.ascontiguousarray(
            k[b, :, 2 * m:2 * m + 2, :].astype(np.float16)
            .transpose(2, 1, 0))                       # [D, 2, S]
        vc = v[b, :, 2 * m:2 * m + 2, :].astype(np.float16)
        vaug = np.ones((S, 2, D + 1), dtype=np.float16)
        vaug[:, :, :D] = vc
        vaug = np.ascontiguousarray(
            vaug.reshape(NT, TS, 2, D + 1).transpose(1, 0, 2, 3))  # [TS,NT,2,D+1]
        in_maps.append({
            "qT": qT, "kT": kT, "vaug": vaug, "maskb": masks_f16[b],
        })
    return in_maps


def kernel(q, k, v, bidirectional_mask, chunk_size):
    q = np.asarray(q, dtype=np.float32)
    k = np.asarray(k, dtype=np.float32)
    v = np.asarray(v, dtype=np.float32)
    chunk = int(np.asarray(chunk_size))

    allowed_T = _allowed_T(bidirectional_mask, chunk)
    sched = Schedule(allowed_T)
    nc = _get_kernel(sched)

    masks_f16 = [sched.mask_data(allowed_T[b]) for b in range(B)]
    in_maps = _shard_inputs(q, k, v, masks_f16)

    res = run_bass_kernel_spmd(nc, in_maps, list(range(N_CORES)))

    out = np.empty((B, S, HQ, D), dtype=np.float32)
    for core in range(N_CORES):
        b = core // 4
        m = core % 4
        oc = res.results[core]["o"]      # [TS, N_GROUPS, GROUP_SUBTILES, 4, D]
        oc = oc.transpose(1, 2, 0, 3, 4).reshape(S, HEADS_PER_CORE, D)
        out[b, :, 4 * m:4 * m + 4, :] = oc.astype(np.float32)
    return out
